# revision 30
# baseline (speedup 1.0000x reference)
"""MCKRL (multi-channel RGCN + semantic attention) Trainium2 kernel.

Strategy (8 NeuronCores, 1 chip):
  - Channels {0,2,3} of the reference contribute to the output (channel 1 is
    computed but unused by the reference stack) -> skip channel 1.
  - Channel c -> a pair of cores {2i, 2i+1}; each core handles one half of the
    destination nodes for every conv of the channel. Core pair 3 duplicates
    channel 0 (spare). In solo mode each core does a whole channel.
  - graph_conv(x, s, d, W, b) = diag(deg_in^-1/2) A diag(deg_out^-1/2) (x W) + b
    is computed as: GEMM (x @ W, scaled per-node on PSUM evict) -> bf16 table
    in HBM -> dma_gather of edge source rows (dst-sorted, degree-binned,
    slice-major padded) -> strided tensor_reduce segment sum -> per-partition
    scale by deg_in^-1/2, + b, pairwise add across the two incoming relations,
    relu.
  - Host-side preprocessing: edge sorting/CSR, degree rsqrt vectors, index
    wrapping for the DMA gather unit. Host-side postprocessing: un-permute,
    softmax over 3 scalars (betas), beta-weighted sum of the 3 per-channel
    partial embeddings (the channel-sharded "unshard" step).
"""
import numpy as np
import sys

sys.path.insert(0, "/opt/trn_rl_repo")

N = 30000          # nodes per type
E = 250000         # edges per (channel, relation)
IN_F, HID_F, OUT_F = 512, 256, 128
NPAD = 30720       # 240 * 128
TROW1 = NPAD + 128  # table rows (+zero row block), L1 & L2 tables
NT = NPAD // 128   # 240 dst tiles per type
CHANNELS = (0, 2, 3)
N_CORES = 8
GATHER_BATCH = 4   # dst tiles per dma_gather call

_compiled = {}


# ----------------------------------------------------------------- host prep
def _prep_channel(c, src, dst, rng_pad=0):
    """Preprocess one channel: permutations, index blocks, degree vectors.

    Returns dict of host arrays for this channel.
    src, dst: [4, E] int32 for relations 0..3 of channel c.
    """
    out = {}
    # degrees (out: over src, in: over dst), clamped to >= 1
    deg_out = np.zeros((4, N), np.float32)
    deg_in = np.zeros((4, N), np.float32)
    for r in range(4):
        deg_out[r] = np.maximum(np.bincount(src[r], minlength=N), 1.0)
        deg_in[r] = np.maximum(np.bincount(dst[r], minlength=N), 1.0)
    s_out = deg_out ** -0.5   # [4, N]
    s_in = deg_in ** -0.5

    # dst-type of relation r: 0->drug, 1->pro, 2->drug, 3->pro
    # perm for drug-dst tiles shared by relations (0, 2); pro by (1, 3).
    perms = {}
    for t, (ra, rb) in enumerate(((0, 2), (1, 3))):
        key = np.maximum(np.bincount(dst[ra], minlength=N),
                         np.bincount(dst[rb], minlength=N))
        perm = np.argsort(-key, kind="stable").astype(np.int32)  # desc degree
        perm = np.concatenate([perm, np.arange(N, NPAD, dtype=np.int32)])
        perms[t] = perm
    out["perm_d"] = perms[0]
    out["perm_p"] = perms[1]

    # For layer 2 gathers the src ids must be table-row positions. L2 table
    # rows are in perm1 order of the src type.
    pos_d = np.empty(NPAD, np.int32)
    pos_d[perms[0]] = np.arange(NPAD, dtype=np.int32)
    pos_p = np.empty(NPAD, np.int32)
    pos_p[perms[1]] = np.arange(NPAD, dtype=np.int32)

    # per relation: sort edges by (perm position of dst)
    def build_idx(r, perm, l2):
        dperm = np.empty(NPAD, np.int64)
        dperm[perm] = np.arange(NPAD)
        key = dperm[dst[r]]                      # perm position of dst
        order = np.argsort(key, kind="stable")
        s_sorted = src[r][order]
        if l2:  # remap src ids to L2 table row positions
            pos = pos_d if r in (0, 1) else pos_p
            s_sorted = pos[s_sorted]
        k_sorted = key[order]
        counts = np.bincount(k_sorted, minlength=NPAD)   # deg per perm pos
        starts = np.zeros(NPAD + 1, np.int64)
        np.cumsum(counts, out=starts[1:])
        # per tile max degree
        ct = counts.reshape(NT, 128)
        Dt = ct.max(axis=1)
        idx_blocks = []
        for t in range(NT):
            D = int(Dt[t])
            if D == 0:
                idx_blocks.append(np.zeros((0,), np.int16))
                continue
            blk = np.full((D, 128), NPAD, np.int32)  # pad -> zero row
            base = t * 128
            for p in range(128):
                st, en = starts[base + p], starts[base + p + 1]
                d = en - st
                if d:
                    blk[:d, p] = s_sorted[st:en]
            idx_blocks.append(blk.reshape(-1).astype(np.int16))
        return idx_blocks, Dt.astype(np.int64), counts

    for r in range(4):
        perm = perms[0] if r in (0, 2) else perms[1]
        blocks, Dt, _ = build_idx(r, perm, l2=False)
        out[f"idx1_{r}"] = blocks
        out[f"D1_{r}"] = Dt
        blocks, Dt, _ = build_idx(r, perm, l2=True)
        out[f"idx2_{r}"] = blocks
        out[f"D2_{r}"] = Dt
        # scale vectors in useful layouts
        sol = np.zeros(NPAD, np.float32)
        sol[:N] = s_out[r]
        out[f"s_out1_{r}"] = sol                      # natural order (L1 table)
        src_perm = perms[0] if r in (0, 1) else perms[1]  # src type of r
        so2 = np.zeros(NPAD, np.float32)
        so2[:N] = s_out[r]
        out[f"s_out2_{r}"] = so2[src_perm]            # perm1 order (L2 table)
        sil = np.zeros(NPAD, np.float32)
        sil[:N] = s_in[r]
        out[f"s_in1_{r}"] = sil[perm]                 # perm order per dst tile
        out[f"s_in2_{r}"] = sil[perm]
    return out


def _wrap_idx(flat):
    """int16 flat index list (len % 128 == 0) -> [128, len/16] wrapped+replicated."""
    L = flat.shape[0]
    w = flat.reshape(L // 16, 16).T  # [16, L/16]
    return np.tile(w, (8, 1)).copy()


# ------------------------------------------------------------- device build
def _build_program(D1, D2, solo, with_bias=True):
    """Build the SPMD program. D1/D2: [4][NT] compile-time slice counts
    (max over active cores). solo: every core does all NT tiles of its
    channel; else half."""
    import concourse.bacc as bacc
    import concourse.mybir as mybir
    from concourse import tile

    BF = mybir.dt.bfloat16
    F32 = mybir.dt.float32
    I16 = mybir.dt.int16

    nt_core = NT if solo else NT // 2
    nch_core = NPAD // 128 if solo else NPAD // 256  # gemm node chunks

    idx1_len = [int(128 * D1[r][:NT].sum()) for r in range(4)]
    idx2_len = [int(128 * D2[r][:NT].sum()) for r in range(4)]

    import os as _os
    nc = bacc.Bacc("TRN2", target_bir_lowering=False, debug=False,
                   num_devices=N_CORES,
                   dynamic_dma_scratch_size=int(_os.environ.get(
                       "MCKRL_DDS", "65536")),
                   num_swdge_queues=int(_os.environ.get("MCKRL_NSQ", "1")))

    # ---------------- inputs
    xdT = nc.dram_tensor("xdT", [IN_F, nch_core * 128], BF, kind="ExternalInput")
    xpT = nc.dram_tensor("xpT", [IN_F, nch_core * 128], BF, kind="ExternalInput")
    W1 = nc.dram_tensor("W1", [4, IN_F, HID_F], BF, kind="ExternalInput")
    W2 = nc.dram_tensor("W2", [4, HID_F, OUT_F], BF, kind="ExternalInput")
    Wp = nc.dram_tensor("Wp", [OUT_F, OUT_F], BF, kind="ExternalInput")
    bp_rep = nc.dram_tensor("bp_rep", [128, OUT_F], F32, kind="ExternalInput")
    q_rep = nc.dram_tensor("q_rep", [128, OUT_F], F32, kind="ExternalInput")
    b1_rep = nc.dram_tensor("b1_rep", [2, 128, HID_F], F32, kind="ExternalInput")
    b2_rep = nc.dram_tensor("b2_rep", [2, 128, OUT_F], F32, kind="ExternalInput")
    # scale vectors: s_out1 [4, nch, 128]; s_in1/s_in2 [4, nt_core, 128];
    # s_out2 [4, nch, 128]
    s_out1 = nc.dram_tensor("s_out1", [4, nch_core, 128], F32, kind="ExternalInput")
    s_out2 = nc.dram_tensor("s_out2", [4, nch_core, 128], F32, kind="ExternalInput")
    s_in1 = nc.dram_tensor("s_in1", [4, nt_core, 128], F32, kind="ExternalInput")
    s_in2 = nc.dram_tensor("s_in2", [4, nt_core, 128], F32, kind="ExternalInput")
    idx1 = [nc.dram_tensor(f"idx1_{r}", [128, max(idx1_len[r], 2048) // 16], I16,
                           kind="ExternalInput") for r in range(4)]
    idx2 = [nc.dram_tensor(f"idx2_{r}", [128, max(idx2_len[r], 2048) // 16], I16,
                           kind="ExternalInput") for r in range(4)]

    # ---------------- outputs (per core): its h2 rows + w partial sums
    h2_out = nc.dram_tensor("h2_out", [2, nt_core * 128, OUT_F], F32,
                            kind="ExternalOutput")
    w_out = nc.dram_tensor("w_out", [128, 2], F32, kind="ExternalOutput")

    # ---------------- internal DRAM
    tabs1 = [nc.dram_tensor(f"tab1_{r}", [TROW1, HID_F], BF) for r in range(4)]
    tabs2 = [nc.dram_tensor(f"tab2_{r}", [TROW1, OUT_F], BF) for r in range(4)]
    h_dram = [nc.dram_tensor(f"h_{t}", [nt_core * 128, HID_F], BF)
              for t in range(2)]  # combined L1 output per dst type (my rows)

    SRC_OF = (0, 0, 1, 1)   # src type of relation r (0=drug,1=pro)
    DST_OF = (0, 1, 0, 1)   # dst type

    with tile.TileContext(nc) as tc:
        with (
            tc.tile_pool(name="const", bufs=1) as cpool,
            tc.tile_pool(name="w", bufs=1) as wpool,
            tc.tile_pool(name="gemm", bufs=3) as gpool,
            tc.tile_pool(name="gath", bufs=int(_os.environ.get("MCKRL_GBUFS", "2"))) as gapool,
            tc.tile_pool(name="acc", bufs=4) as apool,
            tc.tile_pool(name="psum", bufs=2, space="PSUM") as pspool,
            tc.tile_pool(name="psum2", bufs=2, space="PSUM") as pspool2,
        ):
            # ---- constants to SBUF
            w1_sb = wpool.tile([128, 4 * 4 * HID_F], BF, tag="w1")
            for r in range(4):
                nc.sync.dma_start(
                    out=w1_sb[:, r * 4 * HID_F:(r + 1) * 4 * HID_F]
                        .rearrange("p (k f) -> p k f", k=4),
                    in_=W1[r].rearrange("(k p) f -> p k f", p=128))
            w2_sb = wpool.tile([128, 4 * 2 * OUT_F], BF, tag="w2")
            for r in range(4):
                nc.sync.dma_start(
                    out=w2_sb[:, r * 2 * OUT_F:(r + 1) * 2 * OUT_F]
                        .rearrange("p (k f) -> p k f", k=2),
                    in_=W2[r].rearrange("(k p) f -> p k f", p=128))
            wp_sb = wpool.tile([128, OUT_F], BF, tag="wp")
            nc.sync.dma_start(out=wp_sb[:], in_=Wp[:, :])
            bp_sb = cpool.tile([128, OUT_F], F32, tag="bp")
            nc.sync.dma_start(out=bp_sb[:], in_=bp_rep[:, :])
            q_sb = cpool.tile([128, OUT_F], F32, tag="q")
            nc.sync.dma_start(out=q_sb[:], in_=q_rep[:, :])
            b1_sb = cpool.tile([128, 2 * HID_F], F32, tag="b1")
            nc.sync.dma_start(
                out=b1_sb[:].rearrange("p (t f) -> p t f", t=2),
                in_=b1_rep.rearrange("t p f -> p t f"))
            b2_sb = cpool.tile([128, 2 * OUT_F], F32, tag="b2")
            nc.sync.dma_start(
                out=b2_sb[:].rearrange("p (t f) -> p t f", t=2),
                in_=b2_rep.rearrange("t p f -> p t f"))
            so1_sb = cpool.tile([128, 4 * nch_core], F32, tag="so1")
            nc.sync.dma_start(
                out=so1_sb[:].rearrange("p (r m) -> p r m", r=4),
                in_=s_out1.rearrange("r m p -> p r m"))
            so2_sb = cpool.tile([128, 4 * nch_core], F32, tag="so2")
            nc.sync.dma_start(
                out=so2_sb[:].rearrange("p (r m) -> p r m", r=4),
                in_=s_out2.rearrange("r m p -> p r m"))
            si1_sb = cpool.tile([128, 4 * nt_core], F32, tag="si1")
            nc.sync.dma_start(
                out=si1_sb[:].rearrange("p (r m) -> p r m", r=4),
                in_=s_in1.rearrange("r m p -> p r m"))
            si2_sb = cpool.tile([128, 4 * nt_core], F32, tag="si2")
            nc.sync.dma_start(
                out=si2_sb[:].rearrange("p (r m) -> p r m", r=4),
                in_=s_in2.rearrange("r m p -> p r m"))

            # zero rows of the tables
            zrow = cpool.tile([128, HID_F], BF, tag="zrow")
            nc.vector.memset(zrow[:], 0.0)
            for r in range(4):
                nc.sync.dma_start(out=tabs1[r][NPAD:NPAD + 128, :], in_=zrow[:])
                nc.sync.dma_start(out=tabs2[r][NPAD:NPAD + 128, :],
                                  in_=zrow[:, :OUT_F])

            # identity for PE transpose
            from concourse.masks import make_identity
            ident = cpool.tile([128, 128], F32, tag="ident")
            make_identity(nc, ident[:])

            # ---------------- L1 GEMMs: tables tab1_r rows for my node chunks
            for m in range(nch_core):
                for st in range(2):  # src type: 0 drug (r 0,1), 1 pro (r 2,3)
                    xT = xdT if st == 0 else xpT
                    lhs = gpool.tile([128, 4 * 128], BF, tag="lhs1")
                    nc.sync.dma_start(
                        out=lhs[:].rearrange("p (k n) -> p k n", k=4),
                        in_=xT[:, m * 128:(m + 1) * 128]
                            .rearrange("(k p) n -> p k n", p=128))
                    for r in (0 + 2 * st, 1 + 2 * st):
                        ps = pspool.tile([128, HID_F], F32, tag="ps1")
                        for k in range(4):
                            nc.tensor.matmul(
                                out=ps[:],
                                lhsT=lhs[:, k * 128:(k + 1) * 128],
                                rhs=w1_sb[:, (r * 4 + k) * HID_F:
                                          (r * 4 + k + 1) * HID_F],
                                start=(k == 0), stop=(k == 3))
                        ev = gpool.tile([128, HID_F], BF, tag="ev1")
                        nc.scalar.activation(
                            out=ev[:], in_=ps[:],
                            func=mybir.ActivationFunctionType.Copy,
                            scale=so1_sb[:, r * nch_core + m:
                                         r * nch_core + m + 1])
                        nc.sync.dma_start(
                            out=tabs1[r][m * 128:(m + 1) * 128, :], in_=ev[:])

            # ---------------- L1 gather + reduce + combine -> h_dram
            GB = GATHER_BATCH

            def conv_pass(tabs, idx_t, D, F, si_sb, so_next, b_sb, layer):
                """Tiles packed into variable groups (sum of slice counts
                <= CAP): one dma_gather per (relation, group), then per-tile
                reduce + fused combine; layer1 -> h_dram; layer2 -> h2_out
                + w stage."""
                import os as _os2
                CAP = (int(_os2.environ.get('MCKRL_CAP1', '32'))
                       if layer == 1 else
                       int(_os2.environ.get('MCKRL_CAP2', '64')))
                # joint groups per dst type (shared by its two relations)
                groups = {}
                bufsz = {}
                for dt_ in range(2):
                    ra, rb = (0, 2) if dt_ == 0 else (1, 3)
                    gs, t0_, cur = [], 0, 0
                    for t in range(nt_core):
                        d = max(int(D[ra][t]), int(D[rb][t]))
                        if t > t0_ and cur + d > CAP:
                            gs.append((t0_, t - t0_))
                            t0_, cur = t, 0
                        cur += d
                    gs.append((t0_, nt_core - t0_))
                    groups[dt_] = gs
                    bufsz[dt_] = max(
                        max(int(D[r][a:a + n].sum()) for (a, n) in gs)
                        for r in (ra, rb))
                bmax = max(bufsz.values())
                # per (dt, r): per-group index lengths (in 16-col units)
                glen16 = {}
                for dt_ in range(2):
                    ra, rb = (0, 2) if dt_ == 0 else (1, 3)
                    for r in (ra, rb):
                        glen16[r] = [int(D[r][a:a + n].sum()) * 8
                                     for (a, n) in groups[dt_]]
                IW = int(__import__('os').environ.get('MCKRL_IW', '4'))
                ixwin = {}
                ixoff16 = [0, 0, 0, 0]
                for gi in range(max(len(groups[0]), len(groups[1]))):
                    for dt_ in range(2):  # dst type
                        if gi >= len(groups[dt_]):
                            continue
                        t0_, ntl = groups[dt_][gi]
                        ra, rb = (0, 2) if dt_ == 0 else (1, 3)
                        if gi % IW == 0:
                            for r in (ra, rb):
                                wlen = sum(glen16[r][gi:gi + IW])
                                w = gapool.tile([128, bmax * 8 * IW], I16,
                                                tag=f"ix{r % 2}")
                                if wlen:
                                    nc.sync.dma_start(
                                        out=w[:, :wlen],
                                        in_=idx_t[r][:, ixoff16[r]:
                                                     ixoff16[r] + wlen])
                                ixwin[r] = (w, gi, ixoff16[r])
                                ixoff16[r] += wlen
                        gbufs = {}
                        for r in (ra, rb):
                            Lg = 128 * int(D[r][t0_:t0_ + ntl].sum())
                            g = gapool.tile([128, bmax * F], BF,
                                            tag=f"g{r % 2}")
                            gbufs[r] = g
                            if Lg == 0:
                                continue
                            w, wg, _ = ixwin[r]
                            loc = sum(glen16[r][wg:gi])
                            nq = int(__import__("os").environ.get(
                                "MCKRL_NSQ", "1"))
                            nc.gpsimd.dma_gather(
                                out_ap=g[:, :(Lg // 128) * F].rearrange(
                                    "p (d f) -> p d f", f=F),
                                in_ap=tabs[r].ap(),
                                idxs_ap=w[:, loc:loc + Lg // 16],
                                num_idxs=Lg, num_idxs_reg=Lg,
                                elem_size=F, single_packet=False,
                                queue_num=(r % 2) % nq)
                        for tt in range(ntl):
                            t = t0_ + tt
                            accs = {}
                            for r in (ra, rb):
                                Dt = int(D[r][t])
                                acc = apool.tile([128, F], F32,
                                                 tag=f"acc{layer}{r % 2}")
                                if Dt == 0:
                                    nc.vector.memset(acc[:], 0.0)
                                elif __import__("os").environ.get(
                                        "MCKRL_NORED"):
                                    nc.vector.memset(acc[:], 0.0)
                                else:
                                    o = int(D[r][t0_:t].sum()) * F
                                    nc.vector.tensor_reduce(
                                        out=acc[:],
                                        in_=gbufs[r][:, o:o + Dt * F]
                                            .rearrange("p (d f) -> p f d",
                                                       f=F),
                                        axis=mybir.AxisListType.X,
                                        op=mybir.AluOpType.add)
                                accs[r] = acc
                            import os as _os
                            if not _os.environ.get("MCKRL_NOEMIT"):
                                emit_tile(accs, ra, rb, t, dt_, F, si_sb,
                                          b_sb, layer)

            def emit_tile(accs, ra, rb, t, dt_, F, si_sb, b_sb, layer):
                        # combine: relu(acc_ra*si[ra] + acc_rb*si[rb] + b)
                        sc1 = apool.tile([128, F], F32, tag=f"sc{layer}1")
                        nc.scalar.activation(
                            out=sc1[:], in_=accs[rb][:],
                            func=mybir.ActivationFunctionType.Copy,
                            scale=si_sb[:, rb * nt_core + t:
                                        rb * nt_core + t + 1])
                        sc0 = apool.tile([128, F], F32, tag=f"sc{layer}0")
                        nc.vector.scalar_tensor_tensor(
                            out=sc0[:], in0=accs[ra][:],
                            scalar=si_sb[:, ra * nt_core + t:
                                         ra * nt_core + t + 1],
                            in1=sc1[:], op0=mybir.AluOpType.mult,
                            op1=mybir.AluOpType.add)
                        if with_bias:
                            nc.vector.tensor_add(
                                out=sc0[:], in0=sc0[:],
                                in1=b_sb[:, dt_ * F:(dt_ + 1) * F])
                        if layer == 1:
                            h = gpool.tile([128, F], BF, tag="h")
                            nc.scalar.activation(
                                out=h[:], in_=sc0[:],
                                func=mybir.ActivationFunctionType.Relu)
                            nc.sync.dma_start(
                                out=h_dram[dt_][t * 128:(t + 1) * 128, :],
                                in_=h[:])
                        else:
                            z = apool.tile([128, F], F32, tag="z")
                            nc.scalar.activation(
                                out=z[:], in_=sc0[:],
                                func=mybir.ActivationFunctionType.Relu)
                            nc.sync.dma_start(
                                out=h2_out[dt_, t * 128:(t + 1) * 128, :],
                                in_=z[:])
                            # w stage: zT = z^T; tw = tanh(zT.T@Wp + bp);
                            # w = reduce(tw*q); wacc += w
                            pst = pspool2.tile([128, 128], F32, tag="pst")
                            nc.tensor.transpose(out=pst[:], in_=z[:],
                                                identity=ident[:])
                            zT = gpool.tile([128, 128], BF, tag="zT")
                            nc.scalar.copy(out=zT[:], in_=pst[:])
                            psw = pspool2.tile([128, OUT_F], F32, tag="psw")
                            nc.tensor.matmul(out=psw[:], lhsT=zT[:],
                                             rhs=wp_sb[:], start=True,
                                             stop=True)
                            tw = apool.tile([128, OUT_F], F32, tag="tw")
                            nc.vector.tensor_add(out=tw[:], in0=psw[:],
                                                 in1=bp_sb[:])
                            nc.scalar.activation(
                                out=tw[:], in_=tw[:],
                                func=mybir.ActivationFunctionType.Tanh)
                            nc.vector.tensor_mul(out=tw[:], in0=tw[:],
                                                 in1=q_sb[:])
                            wv = apool.tile([128, 1], F32, tag="wv")
                            nc.vector.tensor_reduce(
                                out=wv[:], in_=tw[:],
                                axis=mybir.AxisListType.X,
                                op=mybir.AluOpType.add)
                            nc.vector.tensor_add(
                                out=wacc[:, dt_:dt_ + 1],
                                in0=wacc[:, dt_:dt_ + 1], in1=wv[:])

            conv_pass(tabs1, idx1, D1, HID_F, si1_sb, None, b1_sb, layer=1)

            # ---------------- L2 GEMMs: tab2_r from h_dram
            for m in range(nt_core):
                for st in range(2):
                    lhs = gpool.tile([128, 2 * 128], BF, tag="lhs2")
                    for k in range(2):
                        nc.sync.dma_start(
                            out=lhs[:, k * 128:(k + 1) * 128],
                            in_=h_dram[st][m * 128:(m + 1) * 128,
                                           k * 128:(k + 1) * 128],
                            transpose=True)
                    for r in (0 + 2 * st, 1 + 2 * st):
                        ps = pspool.tile([128, OUT_F], F32, tag="ps2")
                        for k in range(2):
                            nc.tensor.matmul(
                                out=ps[:],
                                lhsT=lhs[:, k * 128:(k + 1) * 128],
                                rhs=w2_sb[:, (r * 2 + k) * OUT_F:
                                          (r * 2 + k + 1) * OUT_F],
                                start=(k == 0), stop=(k == 1))
                        ev = gpool.tile([128, OUT_F], BF, tag="ev2")
                        nc.scalar.activation(
                            out=ev[:], in_=ps[:],
                            func=mybir.ActivationFunctionType.Copy,
                            scale=so2_sb[:, r * nch_core + m:
                                         r * nch_core + m + 1])
                        nc.sync.dma_start(
                            out=tabs2[r][m * 128:(m + 1) * 128, :], in_=ev[:])

            wacc = cpool.tile([128, 2], F32, tag="wacc")
            nc.vector.memset(wacc[:], 0.0)

            conv_pass(tabs2, idx2, D2, OUT_F, si2_sb, None, b2_sb, layer=2)

            nc.sync.dma_start(out=w_out[:, :], in_=wacc[:])

    nc.compile()
    return nc


# ----------------------------------------------------------------- kernel()
def kernel(x_drug, x_pro, src, dst, W1, b1, W2, b2, Wp, bp, q):
    import ml_dtypes
    from concourse.bass_utils import run_bass_kernel_spmd

    solo = True
    x_drug = np.asarray(x_drug, np.float32)
    x_pro = np.asarray(x_pro, np.float32)
    src = np.asarray(src)
    dst = np.asarray(dst)

    # ---- per-channel preprocessing
    preps = {c: _prep_channel(c, src[c], dst[c]) for c in CHANNELS}

    # compile-time D vectors: max across channels
    D1 = [np.maximum.reduce([preps[c][f"D1_{r}"] for c in CHANNELS])
          for r in range(4)]
    D2 = [np.maximum.reduce([preps[c][f"D2_{r}"] for c in CHANNELS])
          for r in range(4)]

    with_bias = bool(np.any(np.asarray(b1)) or np.any(np.asarray(b2)))
    key = ("prog", tuple(int(d.sum()) for d in D1),
           tuple(int(d.sum()) for d in D2), solo, with_bias)
    if key not in _compiled:
        _compiled[key] = _build_program(D1, D2, solo, with_bias)
    nc = _compiled[key]

    # pad idx blocks to compile-time D and concatenate
    def pack_idx(prep, r, D, l2):
        blocks = prep[f"idx2_{r}" if l2 else f"idx1_{r}"]
        out_blocks = []
        for t in range(NT):
            Dt = int(D[r][t])
            blk = blocks[t]
            full = np.full((Dt * 128,), NPAD, np.int16)
            full[:blk.shape[0]] = blk
            out_blocks.append(full)
        flat = np.concatenate(out_blocks) if out_blocks else \
            np.zeros((0,), np.int16)
        L = flat.shape[0]
        tgt = max(L, 2048)
        if L < tgt:
            flat = np.concatenate([flat, np.full(tgt - L, NPAD, np.int16)])
        return _wrap_idx(flat)

    # host transposes of inputs (layout prep), bf16
    xdT = np.zeros((IN_F, NPAD), ml_dtypes.bfloat16)
    xdT[:, :N] = x_drug.T.astype(ml_dtypes.bfloat16)
    xpT = np.zeros((IN_F, NPAD), ml_dtypes.bfloat16)
    xpT[:, :N] = x_pro.T.astype(ml_dtypes.bfloat16)

    nch = NPAD // 128
    ntc = NT

    in_maps = []
    for core in range(N_CORES):
        c = CHANNELS[core % len(CHANNELS)]
        p = preps[c]
        im = {
            "xdT": xdT, "xpT": xpT,
            "W1": np.asarray(W1[c], np.float32).astype(ml_dtypes.bfloat16),
            "W2": np.asarray(W2[c], np.float32).astype(ml_dtypes.bfloat16),
            "Wp": np.asarray(Wp, np.float32).astype(ml_dtypes.bfloat16),
            "bp_rep": np.tile(np.asarray(bp, np.float32)[None, :], (128, 1)),
            "q_rep": np.tile(np.asarray(q, np.float32)[None, :], (128, 1)),
            "b1_rep": np.stack([
                np.tile((np.asarray(b1[c][0], np.float32)
                         + np.asarray(b1[c][2], np.float32))[None, :], (128, 1)),
                np.tile((np.asarray(b1[c][1], np.float32)
                         + np.asarray(b1[c][3], np.float32))[None, :], (128, 1)),
            ]),
            "b2_rep": np.stack([
                np.tile((np.asarray(b2[c][0], np.float32)
                         + np.asarray(b2[c][2], np.float32))[None, :], (128, 1)),
                np.tile((np.asarray(b2[c][1], np.float32)
                         + np.asarray(b2[c][3], np.float32))[None, :], (128, 1)),
            ]),
            "s_out1": np.stack([p[f"s_out1_{r}"].reshape(nch, 128)
                                for r in range(4)]),
            "s_out2": np.stack([p[f"s_out2_{r}"].reshape(nch, 128)
                                for r in range(4)]),
            "s_in1": np.stack([p[f"s_in1_{r}"].reshape(ntc, 128)
                               for r in range(4)]),
            "s_in2": np.stack([p[f"s_in2_{r}"].reshape(ntc, 128)
                               for r in range(4)]),
        }
        for r in range(4):
            im[f"idx1_{r}"] = pack_idx(p, r, D1, l2=False)
            im[f"idx2_{r}"] = pack_idx(p, r, D2, l2=True)
        in_maps.append(im)

    import os
    if os.environ.get("MCKRL_SIM"):
        from concourse.bass_interp import MultiCoreSim
        sim = MultiCoreSim(nc, num_cores=N_CORES, trace=False)
        for i in range(N_CORES):
            for k, v in in_maps[i].items():
                sim.cores[i].tensor(k)[:] = v
        sim.simulate(check_with_hw=False)
        results = [{"h2_out": np.asarray(sim.cores[i].mem_tensor("h2_out")),
                    "w_out": np.asarray(sim.cores[i].mem_tensor("w_out"))}
                   for i in range(N_CORES)]
    else:
        res = run_bass_kernel_spmd(nc, in_maps, core_ids=list(range(N_CORES)))
        results = res.results

    # ---- host postprocessing: unpermute, betas, weighted channel sum
    h2 = {}
    wsum = {}
    for i, c in enumerate(CHANNELS):
        r = results[i]
        out = np.asarray(r["h2_out"]).reshape(2, -1, OUT_F)
        pd, pp = preps[c]["perm_d"], preps[c]["perm_p"]
        hd = np.empty((NPAD, OUT_F), np.float32)
        hd[pd] = out[0]
        hp = np.empty((NPAD, OUT_F), np.float32)
        hp[pp] = out[1]
        h2[c] = (hd[:N], hp[:N])
        wsum[c] = np.asarray(r["w_out"]).reshape(128, 2).sum(axis=0)

    w_mean = {c: wsum[c] / N for c in CHANNELS}
    wd = np.array([w_mean[c][0] for c in CHANNELS])
    wp_ = np.array([w_mean[c][1] for c in CHANNELS])

    def softmax(v):
        e = np.exp(v - v.max())
        return e / e.sum()

    beta_d = softmax(wd).astype(np.float32)
    beta_p = softmax(wp_).astype(np.float32)

    emb_d = np.zeros((N, OUT_F), np.float32)
    emb_p = np.zeros((N, OUT_F), np.float32)
    for k, c in enumerate(CHANNELS):
        emb_d += beta_d[k] * h2[c][0]
        emb_p += beta_p[k] * h2[c][1]

    return emb_d, emb_p, beta_d, beta_p
